# revision 3
# baseline (speedup 1.0000x reference)
"""Trainium2 Bass kernel for nn_AttnCorrelation_63007170232539.

Reference math:
    q = conv1x1(feat1); k = conv1x1(feat2)
    for each of 81 shifts: corr = mean_c(q * shift(k))  -> [B, 1, H, W]
    out_shift = softmax(corr, axis=1)[:, 0]             -> [B, H, W]

The softmax is taken over an axis of size 1, so for every finite input the
output is identically 1.0 — the function is constant on its domain (and the
inputs here cannot overflow to inf/nan: |conv out| <= ~sqrt(C)*max|W|*max|x|).
The kernel therefore reduces to producing ones((B, 81, H, W)) as fast as the
hardware can write it.

Sharding: pure data parallel — each of the 8 cores produces one batch
element's [81, 96, 128] f32 slice (3.98 MB).  Per core: DVE memsets one
[128, 486] SBUF tile to 1.0 (~0.25 MB), then 16 HWDGE DMAs fan it out to
the DRAM output viewed as [128, 7776].  Cost model (TimelineSim, production
InstructionCostModel): ~15.4 us/core = 2.1 us fixed + 2.2 us DMA latency +
11.1 us of data at the ~358 GB/s HBM-per-core write limit, i.e. the kernel
sits on the memory roofline.
"""

import time

import numpy as np

B, NSQ, H, W = 8, 81, 96, 128
PER_CORE = NSQ * H * W  # 995328 = 128 * 7776
P = 128
FREE = PER_CORE // P  # 7776
CHUNK = 486  # 16 DMAs x 128*486*4B = 243 KB each, all reading one SBUF tile
N_CORES = 8

_cached = {}


def _build():
    import concourse.bass as bass
    from concourse import mybir

    nc = bass.Bass()
    out = nc.declare_dram_parameter("out", [P, FREE], mybir.dt.float32, isOutput=True)
    n_dma = FREE // CHUNK
    with (
        nc.Block(no_gpsimd_drain=True) as block,
        nc.semaphore("ms_sem") as ms_sem,
        nc.semaphore("dma_sem") as dma_sem,
        nc.sbuf_tensor("ones", [P, CHUNK], mybir.dt.float32) as ones,
    ):

        @block.vector
        def _(vector):
            vector.memset(ones[:], 1.0).then_inc(ms_sem, 1)

        @block.sync
        def _(sync):
            sync.wait_ge(ms_sem, 1)
            for i in range(n_dma):
                sync.dma_start(
                    out=out[:, i * CHUNK : (i + 1) * CHUNK], in_=ones[:]
                ).then_inc(dma_sem, 16)
            sync.wait_ge(dma_sem, 16 * n_dma)

    return nc


def kernel(**inputs) -> np.ndarray:
    from concourse.bass_utils import run_bass_kernel_spmd

    if "nc" not in _cached:
        _cached["nc"] = _build()
    nc = _cached["nc"]
    core_ids = list(range(N_CORES))
    in_maps = [{} for _ in core_ids]
    last_err = None
    for attempt in range(3):
        try:
            res = run_bass_kernel_spmd(nc, in_maps, core_ids)
            break
        except Exception as e:  # transient NRT/device errors: retry
            last_err = e
            print(f"kernel: attempt {attempt} failed ({e}); retrying", flush=True)
            time.sleep(2.0)
    else:
        raise last_err
    outs = [np.asarray(r["out"]).reshape(NSQ, H, W) for r in res.results]
    return np.stack(outs).astype(np.float32, copy=False)


if __name__ == "__main__":
    out = kernel()
    print(out.shape, out.dtype, out.min(), out.max())
    print("all ones:", np.all(out == 1.0))
